# revision 1
# baseline (speedup 1.0000x reference)
"""Trainium2 Bass kernel for nn_DeepGCN_slic_V2 (segment_reduce).

kernel(segmaps, conv_feat) -> [64, 768, 14, 14] float32

Sharding: pure data parallel over batch, 8 samples per NeuronCore x 8 cores.

Host prep (cheap numpy, part of sharding): slice segmaps to the corner
rows/cols ({1,2} mod 4 -> [8,112,112] per core), transpose conv_feat to
[pq, f] layout and split into bf16 hi/lo halves (exact fp32 reconstruction
inside the fp32 PSUM accumulation; same total DMA bytes as fp32).

On-chip algorithm per core (8 samples at once where possible):
  * cell masks: for each of 56x56 cells, OR of the 4 corner one-hots,
    bit-packed into 8 int32 words (26 bins/word, shift headroom 4 bits)
  * 4x4 window sums in the bit domain via bit-sliced adder trees
    -> 5 bit-planes of wcnt[window, bin]
  * tiny DRAM-hop transpose to put windows (pq) on partitions
  * arithmetic unpack to wcnt [98 x (16 chunk, 256)] bf16 (exact ints 0..16)
  * PE matmuls: pooled[f, m] = featT.T @ wcnt (bf16 hi+lo, fp32 PSUM),
    counts[m] via ones-matmul
  * out = pooled * broadcast(1 / max(counts, 1)) on DVE, DMA to DRAM
"""
import sys
sys.path.insert(0, "/opt/trn_rl_repo")
from contextlib import ExitStack

import numpy as np
import ml_dtypes

import concourse.tile as tile
from concourse import mybir, bacc
from concourse.bass_utils import run_bass_kernel_spmd

Alu = mybir.AluOpType
f32 = mybir.dt.float32
i32 = mybir.dt.int32
bf16 = mybir.dt.bfloat16
XY = mybir.AxisListType.XY

S = 8          # samples per core
PW = 14        # window rows/cols
NSEG = 196
NW = 8         # int32 words per bin-vector
BPW = 26       # bins per word (max bit 25+4=29: headroom for <<4, no sign bit)
M = NW * 32    # 256: internal padded bin columns (32 slots/word, 26 used)
LV = NW * BPW  # 208: label-ordered view width
C = 768
P112 = S * PW  # 112 partitions: (s, p)
FREE = 896     # free size of seg tile: (isub 4, dr 2, col 112)


def _emit_kernel(ctx, nc, tc, seg, featH, featL, out):
    dve, gp, pe = nc.vector, nc.gpsimd, nc.tensor

    cpool = ctx.enter_context(tc.tile_pool(name="const", bufs=1))
    spool = ctx.enter_context(tc.tile_pool(name="segp", bufs=1))
    mpool = ctx.enter_context(tc.tile_pool(name="mask", bufs=1))
    epool = ctx.enter_context(tc.tile_pool(name="eqb", bufs=2))
    bpool = ctx.enter_context(tc.tile_pool(name="bstage", bufs=1))
    upool = ctx.enter_context(tc.tile_pool(name="unpack", bufs=3))
    wpool = ctx.enter_context(tc.tile_pool(name="wcnt", bufs=1))
    fpool = ctx.enter_context(tc.tile_pool(name="feat", bufs=4))
    opool = ctx.enter_context(tc.tile_pool(name="outb", bufs=6))
    vpool = ctx.enter_context(tc.tile_pool(name="inv", bufs=2))
    pscnt = ctx.enter_context(tc.tile_pool(name="pscnt", bufs=2, space="PSUM"))
    psmm = ctx.enter_context(tc.tile_pool(name="psmm", bufs=6, space="PSUM"))

    # ---- constants ----
    ones_i = cpool.tile([P112, 1], i32)
    gp.memset(ones_i[:], 1)
    shamt = cpool.tile([98, LV], i32)
    gp.iota(shamt[:], pattern=[[0, NW], [1, BPW]], base=0, channel_multiplier=0)
    ones_b = cpool.tile([98, 1], bf16)
    gp.memset(ones_b[:], 1.0)
    plane_c = []
    for pl in range(5):
        pc = cpool.tile([98, 1], i32, name=f"plc{pl}")
        gp.memset(pc[:], 1 << pl)
        plane_c.append(pc)

    # ---- load seg rows: partition (s,p), free (isub 4, dr 2, col 112) ----
    segt = spool.tile([P112, FREE], mybir.dt.uint8)
    seg_dma = nc.sync.dma_start(
        segt[:], seg.rearrange("s (p r) c -> (s p) (r c)", p=PW))

    # ---- A: cell bitmasks mw[112, (w 8, isub 4, jj 56)] ----
    hwm = mpool.tile([P112, FREE], i32)    # L * 631
    dve.tensor_scalar(out=hwm[:], in0=segt[:], scalar1=631, scalar2=None,
                      op0=Alu.mult)
    hw_t = mpool.tile([P112, FREE], i32)   # word index = L // 26
    dve.tensor_scalar(out=hw_t[:], in0=hwm[:], scalar1=14, scalar2=None,
                      op0=Alu.logical_shift_right)
    low_t = mpool.tile([P112, FREE], i32)  # bit index = L - 26 * word
    dve.scalar_tensor_tensor(out=low_t[:], in0=hw_t[:], scalar=-26,
                             in1=segt[:], op0=Alu.mult, op1=Alu.add)
    b_t = mpool.tile([P112, FREE], i32)
    dve.tensor_tensor(out=b_t[:], in0=ones_i[:].to_broadcast((P112, FREE)),
                      in1=low_t[:], op=Alu.logical_shift_left)

    mw = mpool.tile([P112, NW * 224], i32)  # (w 8, isub 4, jj 56)
    eqb = epool.tile([P112, NW * FREE], i32, bufs=1)
    for w in range(NW):
        dve.scalar_tensor_tensor(out=eqb[:, w * FREE:(w + 1) * FREE],
                                 in0=hw_t[:], scalar=w, in1=b_t[:],
                                 op0=Alu.is_equal, op1=Alu.mult)
    # OR over the 4 corners (dr, dc) as two pairwise passes (cheaper than
    # one strided 4:1 tensor_reduce)
    ev = eqb[:].rearrange("p (w i d jj e) -> p w i d jj e",
                          w=NW, i=4, d=2, jj=56)
    o1 = epool.tile([P112, NW * 4 * 2 * 56], i32, bufs=1)  # OR over dc
    dve.tensor_tensor(out=o1[:].rearrange("p (w i d jj) -> p w i d jj",
                                          w=NW, i=4, d=2).unsqueeze(5),
                      in0=ev[:, :, :, :, :, 0:1], in1=ev[:, :, :, :, :, 1:2],
                      op=Alu.bitwise_or)
    ov1 = o1[:].rearrange("p (w i d jj) -> p w i d jj", w=NW, i=4, d=2)
    dve.tensor_tensor(out=mw[:].rearrange("p (w i jj) -> p w i jj",
                                          w=NW, i=4).unsqueeze(3),
                      in0=ov1[:, :, :, 0:1, :], in1=ov1[:, :, :, 1:2, :],
                      op=Alu.bitwise_or)

    # ---- feat loads (hi/lo), per sample: [98, (c 2, f 768)] bf16 ----
    fh, fl = [], []
    for s in range(S):
        th = fpool.tile([98, 2 * C], bf16, name=f"fh{s}", tag="fh")
        nc.scalar.dma_start(th[:],
                            featH[s].rearrange("(c P) f -> P c f", c=2))
        tl = fpool.tile([98, 2 * C], bf16, name=f"fl{s}", tag="fl")
        nc.scalar.dma_start(tl[:],
                            featL[s].rearrange("(c P) f -> P c f", c=2))
        fh.append(th)
        fl.append(tl)

    # ---- B: sum over jsub (4 cells) -> 3 bit planes R0..R2 [112,(w,isub,q)] ----
    def tt(eng, o, a, b, op):
        eng.tensor_tensor(out=o, in0=a, in1=b, op=op)

    mj = mw[:].rearrange("p (w i q j) -> p w i q j", w=NW, i=4, q=PW, j=4)
    ja, jb, jc, jd = (mj[:, :, :, :, k:k + 1] for k in range(4))
    Bsh = [P112, NW * 4 * PW]
    bt_n = [0]

    def new_b():
        t = bpool.tile(Bsh, i32, name=f"bt{bt_n[0]}", tag=f"bt{bt_n[0]}")
        bt_n[0] += 1
        return t

    def bview(t):
        return t[:].rearrange("p (w i q j) -> p w i q j", w=NW, i=4, j=1)

    s0a, c0a, s0b, c0b = new_b(), new_b(), new_b(), new_b()
    R0, u, t_, R1, w1, w2, R2 = (new_b() for _ in range(7))
    g = dve
    tt(g, bview(s0a), ja, jb, Alu.bitwise_xor)
    tt(g, bview(c0a), ja, jb, Alu.bitwise_and)
    tt(g, bview(s0b), jc, jd, Alu.bitwise_xor)
    tt(g, bview(c0b), jc, jd, Alu.bitwise_and)
    tt(g, bview(R0), bview(s0a), bview(s0b), Alu.bitwise_xor)
    tt(g, bview(u), bview(s0a), bview(s0b), Alu.bitwise_and)
    tt(g, bview(t_), bview(c0a), bview(c0b), Alu.bitwise_xor)
    tt(g, bview(R1), bview(t_), bview(u), Alu.bitwise_xor)
    tt(g, bview(w1), bview(c0a), bview(c0b), Alu.bitwise_and)
    tt(g, bview(w2), bview(u), bview(t_), Alu.bitwise_and)
    tt(g, bview(R2), bview(w1), bview(w2), Alu.bitwise_or)

    # ---- C: sum over isub (4 rows of 3-bit) -> 5 planes Z0..Z4 [112,(w,q)] ----
    Zsh = [P112, NW * PW]

    def zt(name):
        return bpool.tile(Zsh, i32, name=name, tag=name)

    # XY merged: one pass computes X (isub 0+1) and Y (isub 2+3) together.
    # Views [112, (w, pair 2, q)]: pair slices isub with stride 2.
    XYsh = [P112, NW * 2 * PW]

    def xt(name):
        return bpool.tile(XYsh, i32, name=name, tag=name)

    def rpair(t, k):
        # R [112,(w,isub 4,q)] -> [p, w, 2, q]: isub in {k, k+2}
        return t[:].rearrange("p (w i q) -> p w i q", w=NW, i=4)[:, :, k::2, :]

    def xv(t):
        return t[:].rearrange("p (w i q) -> p w i q", w=NW, i=2)

    def add3_pair(prefix):
        pa = [rpair(R0, 0), rpair(R1, 0), rpair(R2, 0)]
        pb = [rpair(R0, 1), rpair(R1, 1), rpair(R2, 1)]
        o = [xt(f"{prefix}{k}") for k in range(4)]
        tt(g, xv(o[0]), pa[0], pb[0], Alu.bitwise_xor)
        cy = xt(f"{prefix}c")
        tt(g, xv(cy), pa[0], pb[0], Alu.bitwise_and)
        for bit in (1, 2):
            t1, m1, m2, ncy = (xt(f"{prefix}x{bit}{k}") for k in range(4))
            tt(g, xv(t1), pa[bit], pb[bit], Alu.bitwise_xor)
            tt(g, xv(o[bit]), xv(t1), xv(cy), Alu.bitwise_xor)
            tt(g, xv(m1), pa[bit], pb[bit], Alu.bitwise_and)
            tt(g, xv(m2), xv(cy), xv(t1), Alu.bitwise_and)
            tt(g, xv(ncy), xv(m1), xv(m2), Alu.bitwise_or)
            cy = ncy
        o[3] = cy
        return o

    def half(t, k):
        # XY tile [112,(w, pair 2, q)] -> [p, w, 1, q] half k (X=0, Y=1)
        return t[:].rearrange("p (w i q) -> p w i q", w=NW, i=2)[:, :, k:k + 1, :]

    def zv(t):
        return t[:].rearrange("p (w q) -> p w q", w=NW).unsqueeze(2)

    def add4(xy, prefix):
        pa = [half(t, 0) for t in xy]
        pb = [half(t, 1) for t in xy]
        o = [zt(f"{prefix}{k}") for k in range(5)]
        tt(g, zv(o[0]), pa[0], pb[0], Alu.bitwise_xor)
        cy = zt(f"{prefix}c0")
        tt(g, zv(cy), pa[0], pb[0], Alu.bitwise_and)
        for bit in (1, 2, 3):
            t1, m1, m2, ncy = (zt(f"{prefix}x{bit}{k}") for k in range(4))
            tt(g, zv(t1), pa[bit], pb[bit], Alu.bitwise_xor)
            tt(g, zv(o[bit]), zv(t1), zv(cy), Alu.bitwise_xor)
            tt(g, zv(m1), pa[bit], pb[bit], Alu.bitwise_and)
            tt(g, zv(m2), zv(cy), zv(t1), Alu.bitwise_and)
            tt(g, zv(ncy), zv(m1), zv(m2), Alu.bitwise_or)
            cy = ncy
        o[4] = cy
        return o

    xy_planes = add3_pair("XY")
    Z = add4(xy_planes, "Z")

    # ---- pre-shift planes into wsh [112, (q 14, pl 5, w 8)] ----
    wsh = bpool.tile([P112, PW * 5 * NW], i32, tag="wsh")
    wv = wsh[:].rearrange("p (q pl w) -> p pl w q", pl=5, w=NW)
    for pl in range(5):
        dve.tensor_scalar(out=wv[:, pl], in0=Z[pl][:].rearrange(
            "p (w q) -> p w q", w=NW), scalar1=pl, scalar2=None,
            op0=Alu.logical_shift_left)

    # ---- DRAM-hop transpose to tp [98, (s 8, c 2, pl 5, w 8)] ----
    scratches = [
        nc.dram_tensor(f"tp_scratch{g}", [7, PW, 2, 2, 5, NW], i32).ap()
        for g in range(4)]
    tp = wpool.tile([98, 16 * 5 * NW], i32)
    tpr = tp[:].rearrange("P (s c pl w) -> P s c pl w", s=S, c=2, pl=5)
    heng = [nc.sync, nc.scalar]

    prev_read = [None]

    def emit_hop(s0, ns):
        for s in range(s0, s0 + ns):
            for c in range(2):
                # group 0 on the HW DGE queues (fast path to the first
                # unpack); later groups via SWDGE so they cannot queue
                # ahead of group 0's read on the HW DGE
                eng_w = heng[(2 * s + c) % 2] if s0 == 0 else nc.gpsimd
                wd = eng_w.dma_start(
                    scratches[s // 2][:, :, s % 2, c, :, :],
                    wsh[s * PW + c * 7: s * PW + (c + 1) * 7, :].rearrange(
                        "p (q pl w) -> p q pl w", pl=5, w=NW),
                )

        if ns == 2:
            rd = nc.sync.dma_start(
                tpr[:, s0:s0 + 2],
                scratches[s0 // 2].rearrange(
                    "pp q s c pl w -> (pp q) s c pl w"))
        else:
            rd = nc.sync.dma_start(
                tpr[:, s0:s0 + 1],
                scratches[s0 // 2].rearrange(
                    "pp q s c pl w -> (pp q) s c pl w")[:, s0 % 2:s0 % 2 + 1])
        prev_read[0] = rd


    # ---- unpack + per-sample tail, pipelined in uneven sample groups ----
    GRPS = [(0, 2), (2, 2), (4, 2), (6, 1), (7, 1)]  # (first sample, n samples)
    ov = out.rearrange("s f m -> (s f) m")
    act = nc.scalar

    tpv = tp[:].rearrange("P (s c pl w) -> P (s c) pl w", s=S, c=2, pl=5)
    for gi, (s0, ns) in enumerate(GRPS):
        emit_hop(s0, ns)
        nsc = 2 * ns
        scs = slice(2 * s0, 2 * s0 + nsc)
        shv = shamt[:].rearrange("P (w b) -> P w b", w=NW).unsqueeze(1) \
            .to_broadcast((98, nsc, NW, BPW))
        GL = nsc * LV
        acc = None
        for pl in range(5):
            t_pl = upool.tile([98, GL], i32, name=f"ts{gi}{pl}", tag=f"tshift{ns}")
            tin = tpv[:, scs, pl, :].unsqueeze(3).to_broadcast((98, nsc, NW, BPW))
            dve.tensor_tensor(
                out=t_pl[:].rearrange("P (sc w b) -> P sc w b", sc=nsc, w=NW),
                in0=tin, in1=shv, op=Alu.logical_shift_right)
            nacc = upool.tile([98, GL], i32, name=f"ac{gi}{pl}", tag=f"acc{ns}")
            if pl == 0:
                dve.tensor_scalar(out=nacc[:], in0=t_pl[:], scalar1=1,
                                  scalar2=None, op0=Alu.bitwise_and)
            else:
                dve.scalar_tensor_tensor(out=nacc[:], in0=t_pl[:],
                                         scalar=plane_c[pl][:], in1=acc[:],
                                         op0=Alu.bitwise_and,
                                         op1=Alu.bitwise_or)
            acc = nacc
        wcnt_b = wpool.tile([98, GL], bf16, name=f"wc{gi}", tag=f"wcnt{ns}",
                            bufs=2)
        dve.tensor_copy(out=wcnt_b[:], in_=acc[:])

        for si in range(ns):
            s = s0 + si
            rhs = [wcnt_b[:, (2 * si + c) * LV:(2 * si + c + 1) * LV]
                   for c in range(2)]
            cnt = pscnt.tile([1, LV], f32, tag="cnt")
            pe.matmul(cnt[:], ones_b[:], rhs[0], start=True, stop=False)
            pe.matmul(cnt[:], ones_b[:], rhs[1], start=False, stop=True)
            safe = vpool.tile([1, LV], f32, tag="safe")
            dve.tensor_scalar(out=safe[:], in0=cnt[:], scalar1=1.0,
                              scalar2=None, op0=Alu.max)
            inv = vpool.tile([1, LV], f32, tag="inv")
            dve.reciprocal(inv[:], safe[:])
            invb = vpool.tile([128, LV], f32, tag="invb")
            gp.partition_broadcast(invb[:], inv[:], channels=128)

            fhv = fh[s][:].rearrange("P (c fc f) -> P c fc f", c=2, fc=6)
            flv = fl[s][:].rearrange("P (c fc f) -> P c fc f", c=2, fc=6)
            for fc in range(6):
                pooled = psmm.tile([128, LV], f32, tag="pooled")
                pe.matmul(pooled[:], fhv[:, 0, fc], rhs[0], start=True, stop=False)
                pe.matmul(pooled[:], flv[:, 0, fc], rhs[0], start=False, stop=False)
                pe.matmul(pooled[:], fhv[:, 1, fc], rhs[1], start=False, stop=False)
                pe.matmul(pooled[:], flv[:, 1, fc], rhs[1], start=False, stop=True)
                outb = opool.tile([128, NSEG], f32, tag="outb")
                if gi >= len(GRPS) - 1:
                    dve.tensor_tensor(out=outb[:], in0=pooled[:, 0:NSEG],
                                      in1=invb[:, 0:NSEG], op=Alu.mult)
                else:
                    pevac = opool.tile([128, NSEG], f32, tag="pevac")
                    act.copy(out=pevac[:], in_=pooled[:, 0:NSEG])
                    gp.tensor_tensor(out=outb[:], in0=pevac[:],
                                     in1=invb[:, 0:NSEG], op=Alu.mult)
                nc.sync.dma_start(
                    ov[s * C + fc * 128: s * C + (fc + 1) * 128, :], outb[:])


_NC_CACHE = {}


def _build_module():
    if "nc" in _NC_CACHE:
        return _NC_CACHE["nc"]
    nc = bacc.Bacc("TRN2", target_bir_lowering=False, debug=False,
                   enable_asserts=False)
    seg = nc.dram_tensor("seg", [S, 112, 112], mybir.dt.uint8,
                         kind="ExternalInput").ap()
    featH = nc.dram_tensor("featH", [S, NSEG, C], bf16,
                           kind="ExternalInput").ap()
    featL = nc.dram_tensor("featL", [S, NSEG, C], bf16,
                           kind="ExternalInput").ap()
    out = nc.dram_tensor("out", [S, C, NSEG], f32, kind="ExternalOutput").ap()
    with tile.TileContext(nc) as tc, ExitStack() as ctx:
        _emit_kernel(ctx, nc, tc, seg, featH, featL, out)
    nc.compile()
    _NC_CACHE["nc"] = nc
    return nc


def _host_prep(segmaps, conv_feat):
    segmaps = np.asarray(segmaps)
    conv_feat = np.asarray(conv_feat)
    B = segmaps.shape[0]
    idx = np.array([r for r in range(224) if r % 4 in (1, 2)])
    seg_c = np.ascontiguousarray(segmaps[:, idx][:, :, idx].astype(np.uint8))
    featT = np.ascontiguousarray(
        conv_feat.reshape(B, C, NSEG).transpose(0, 2, 1).astype(np.float32))
    hi = featT.astype(ml_dtypes.bfloat16)
    lo = (featT - hi.astype(np.float32)).astype(ml_dtypes.bfloat16)
    maps = []
    for core in range(8):
        sl = slice(core * S, (core + 1) * S)
        maps.append({"seg": seg_c[sl], "featH": hi[sl], "featL": lo[sl]})
    return maps


def run_on_hw(in_maps, **kwargs):
    nc = _build_module()
    return run_bass_kernel_spmd(nc, in_maps, core_ids=list(range(8)), **kwargs)


def kernel(segmaps, conv_feat):
    in_maps = _host_prep(segmaps, conv_feat)
    res = run_on_hw(in_maps)
    outs = [res.results[c]["out"] for c in range(8)]
    return np.concatenate(outs, 0).reshape(64, C, PW, PW).astype(np.float32)

